# revision 8
# baseline (speedup 1.0000x reference)
"""Bass/Tile kernel for the bidirectional LSTM (S=512, B=64, I=H=512).

Sharding v2 (dir-split): 8 cores = 2 directions x 4 batch-quarters.
Core c: direction d = f if c<4 else b, batch rows [16q : 16q+16], q = c%4.

Per core:
  Phase 1: xproj = x @ W_ih.T + b  (fp32r GEMM) -> DRAM ring, bf16,
           layout ring[step, 64, 512] where rows = chunk-major (c,b):
           gate-column chunk order (f, g, i, o), 512 cols each.
  Phase 2: 512-step recurrence, one chain per core.  All matmuls are plain
    128x128-mode, base partition 0 (no tile_position - col-tiling is a PE
    mode switch that corrupts when mixed with transpose mode).
    PSUM gates tile G [16, 4, 512] fp32 (4 banks, chunk c = bank c):
      - xs inject: matmul(G[:,c,:], eye16, xs[:,c,:], start=True) per chunk
      - 4 hh matmuls bf16 per chunk (k-tiles), accumulate, stop on k=3.
      Chunk-major order so bank c completes early and ACT pipelines behind PE.
    ACT: sigmoid(f), tanh(g), sigmoid(i), sigmoid(o) -> acts4 bf16 base 0;
         tanh(c'); 4 hT psum->sbuf copies.
    DVE: fc = f*c (fp32), ig = g*i (bf16 2x), c' = fc+ig, h = o*tanh(c') per
         128-col chunk; PE transposes h chunks into hT (outstage) for t+1.
    hT lives in outstage [128, 256] (4 steps per slot, 2 slots) -> one output
    DMA per 4 steps into outb [128, 512, 64] bf16.
"""

import sys
if "/opt/trn_rl_repo" not in sys.path:
    sys.path.insert(0, "/opt/trn_rl_repo")
import numpy as np
import ml_dtypes

import concourse.bass as bass
import concourse.bacc as bacc
import concourse.mybir as mybir
import concourse.tile as tile

F32 = mybir.dt.float32
F32R = mybir.dt.float32r
BF16 = mybir.dt.bfloat16
AF = mybir.ActivationFunctionType
NP_BF16 = ml_dtypes.bfloat16

S, B, I, H = 512, 64, 512, 512
NC = 8
BC = 16               # batch rows per core
G4 = 4 * H            # 2048
NKT = 4               # K tiles of 128 over H
NCH = 4               # gate-column chunks of 512
TOK_TILE = 128        # phase-1 token tile = 8 steps
P1_LOOK = 2

# gate-column chunk order (f, g, i, o); PyTorch row order in W is (i, f, g, o)
CHUNK_GATES = (1, 2, 0, 3)
PERM = np.concatenate([np.arange(g * H, (g + 1) * H) for g in CHUNK_GATES])


def prep_core_inputs(inpt, W_ih_f, W_hh_f, b_ih_f, b_hh_f,
                     W_ih_b, W_hh_b, b_ih_b, b_hh_b):
    x_f = np.ascontiguousarray(inpt)
    x_b = np.ascontiguousarray(inpt[::-1])

    per_dir = {}
    for d, (Wih, Whh, bih, bhh) in (("f", (W_ih_f, W_hh_f, b_ih_f, b_hh_f)),
                                    ("b", (W_ih_b, W_hh_b, b_ih_b, b_hh_b))):
        Wr_ih = np.ascontiguousarray(Wih.T[:, PERM], dtype=np.float32)   # [512,2048]
        Wr_hh = np.ascontiguousarray(Whh.T[:, PERM], dtype=np.float32)
        bias = np.ascontiguousarray((bih + bhh)[PERM], dtype=np.float32)[None, :]
        per_dir[d] = {
            "Wih": np.ascontiguousarray(
                Wr_ih.reshape(4, 128, G4).transpose(1, 0, 2)),            # [128,4,2048]
            "Whh": np.ascontiguousarray(
                Wr_hh.reshape(4, 128, G4).transpose(1, 0, 2)).astype(NP_BF16),
            "bias": bias,
        }

    ident = np.eye(BC, dtype=np.float32)
    ones1 = np.ones((1, 128), dtype=np.float32)

    in_maps = []
    for core in range(NC):
        d = "f" if core < 4 else "b"
        q = core % 4
        bs = slice(BC * q, BC * (q + 1))
        x = (x_f if d == "f" else x_b)[:, bs, :]          # [S, 16, 512]
        xT = np.ascontiguousarray(
            x.reshape(S * BC, I).T, dtype=np.float32)     # [512, 8192]
        m = {
            "xT": xT,
            "Wih": per_dir[d]["Wih"],
            "Whh": per_dir[d]["Whh"],
            "bias": per_dir[d]["bias"],
            "ident": ident.astype(NP_BF16),
            "ones1": ones1,
        }
        in_maps.append(m)
    return in_maps


def assemble_output(results):
    out = np.empty((S, B, 2 * H), dtype=np.float32)
    for core in range(NC):
        d = "f" if core < 4 else "b"
        q = core % 4
        bs = slice(BC * q, BC * (q + 1))
        off = 0 if d == "f" else H
        slab = results[core]["outb"].astype(np.float32)    # [128, S, 64]
        h = slab.reshape(128, S, 4, BC).transpose(1, 3, 2, 0).reshape(S, BC, H)
        out[:, bs, off:off + H] = h
    return out


def build_nc(n_steps=S):
    assert n_steps % 8 == 0
    nc = bacc.Bacc("TRN2", target_bir_lowering=False, debug=False)
    n_tt = (n_steps * BC) // TOK_TILE          # token tiles (8 steps each)

    xT_d = nc.declare_dram_parameter("xT", [I, S * BC], F32R, isOutput=False)
    Wih_d = nc.declare_dram_parameter("Wih", [128, 4, G4], F32R, isOutput=False)
    Whh_d = nc.declare_dram_parameter("Whh", [128, 4, G4], BF16, isOutput=False)
    bias_d = nc.declare_dram_parameter("bias", [1, G4], F32R, isOutput=False)
    ident_d = nc.declare_dram_parameter("ident", [BC, BC], BF16, isOutput=False)
    ones_d = nc.declare_dram_parameter("ones1", [1, 128], F32R, isOutput=False)
    outb_d = nc.declare_dram_parameter("outb", [128, n_steps, 4 * BC], BF16,
                                       isOutput=True)
    ring = nc.dram_tensor("ring", [n_steps, BC, 4, 512], BF16)

    with tile.TileContext(nc) as tc:
        with (
            tc.tile_pool(name="wpool", bufs=1) as wpool,
            tc.tile_pool(name="p1x", bufs=2) as p1x,
            tc.tile_pool(name="p1o", bufs=2) as p1o,
            tc.tile_pool(name="p1ps", bufs=2, space="PSUM") as p1ps,
            tc.tile_pool(name="xsp", bufs=4) as xsp,
            tc.tile_pool(name="gps", bufs=1, space="PSUM") as gps,
            tc.tile_pool(name="tps", bufs=2, space="PSUM") as tps,
            tc.tile_pool(name="ep", bufs=2) as ep,
            tc.tile_pool(name="hst", bufs=1) as hst,
        ):
            # resident weights / constants
            Whh = wpool.tile([128, 4, G4], BF16, tag="whh")
            Wih = wpool.tile([128, 4, G4], F32R, tag="wih")
            for k in range(NKT):
                nc.sync.dma_start(Whh[:, k, :], Whh_d[:, k, :])
                nc.sync.dma_start(Wih[:, k, :], Wih_d[:, k, :])
            bias = wpool.tile([1, G4], F32R, tag="bias")
            ident = wpool.tile([BC, BC], BF16, tag="ident")
            ones1 = wpool.tile([1, 128], F32R, tag="ones")
            nc.sync.dma_start(bias[:, :], bias_d[:, :])
            nc.sync.dma_start(ident[:, :], ident_d[:, :])
            nc.sync.dma_start(ones1[:, :], ones_d[:, :])

            # state
            hT0 = hst.tile([128, 4 * BC], BF16, tag="hT0")      # zeros for t=0
            nc.vector.memset(hT0[:, :].bitcast(F32), 0.0)
            ostg = [hst.tile([128, 4 * 4 * BC], BF16, tag=f"ostg{j}",
                             name=f"ostg{j}") for j in range(2)]
            cst = [hst.tile([BC, H], F32, tag=f"cst{j}", name=f"cst{j}")
                   for j in range(2)]
            nc.vector.memset(cst[0][:, :], 0.0)

            xTd = xT_d.rearrange("(k p) t -> p k t", p=128)

            def emit_p1_tile(i):
                xt = p1x.tile([128, 4, TOK_TILE], F32R, tag="xt", name=f"xt{i}")
                nc.sync.dma_start(xt[:, :, :],
                                  xTd[:, :, i * TOK_TILE:(i + 1) * TOK_TILE])
                for c in range(NCH):
                    ps = p1ps.tile([128, 512], F32, tag="p1ps", name=f"p1ps{i}{c}")
                    for k in range(NKT):
                        nc.tensor.matmul(ps[:, :], xt[:, k, :],
                                         Wih[:, k, 512 * c:512 * (c + 1)],
                                         start=(k == 0), stop=False)
                    nc.tensor.matmul(ps[:, :], ones1[:, :],
                                     bias[:, 512 * c:512 * (c + 1)],
                                     start=False, stop=True)
                    xo = p1o.tile([128, 512], BF16, tag="p1o", name=f"p1o{i}{c}")
                    nc.scalar.copy(xo[:, :], ps[:, :])
                    nc.sync.dma_start(
                        ring[8 * i:8 * (i + 1), :, c, :],
                        xo[:, :])

            for i in range(min(P1_LOOK, n_tt)):
                emit_p1_tile(i)

            for t in range(n_steps):
                cur, nxt = t % 2, (t + 1) % 2
                if t % 8 == 0:
                    nxt_tile = t // 8 + P1_LOOK
                    if nxt_tile < n_tt:
                        emit_p1_tile(nxt_tile)

                # previous h (transposed), 64-wide slice of an outstage slot
                if t == 0:
                    hTp = hT0
                    hoff = 0
                else:
                    pj = ((t - 1) // 4) % 2
                    hTp = ostg[pj]
                    hoff = 64 * ((t - 1) % 4)

                xs = xsp.tile([BC, 4, 512], BF16, tag="xs", name=f"xs{t}")
                nc.sync.dma_start(xs[:, :, :], ring[t, :, :, :])

                G = gps.tile([BC, 4, 512], F32, tag="G", name=f"G{t}")
                for c in range(NCH):
                    nc.tensor.matmul(G[:, c, :], ident[:, :], xs[:, c, :],
                                     start=True, stop=False,
                                     skip_group_check=True)
                    for k in range(NKT):
                        nc.tensor.matmul(
                            G[:, c, :],
                            hTp[:, hoff + BC * k:hoff + BC * (k + 1)],
                            Whh[:, k, 512 * c:512 * (c + 1)],
                            start=False, stop=(k == NKT - 1),
                            skip_group_check=True)

                # gate activations: acts4[:, j, :] = f, g, i, o for j = 0..3
                acts4 = ep.tile([BC, 4, 512], BF16, tag="acts", name=f"acts{t}")
                nc.scalar.activation(acts4[:, 0, :], G[:, 0, :], AF.Sigmoid)
                nc.scalar.activation(acts4[:, 1, :], G[:, 1, :], AF.Tanh)
                nc.scalar.activation(acts4[:, 2, :], G[:, 2, :], AF.Sigmoid)
                nc.scalar.activation(acts4[:, 3, :], G[:, 3, :], AF.Sigmoid)

                fc = ep.tile([BC, H], F32, tag="fc", name=f"fc{t}")
                nc.vector.tensor_mul(fc[:, :], acts4[:, 0, :], cst[cur][:, :])
                ig = ep.tile([BC, H], BF16, tag="ig", name=f"ig{t}")
                nc.vector.tensor_mul(ig[:, :], acts4[:, 1, :], acts4[:, 2, :])
                nc.vector.tensor_add(cst[nxt][:, :], fc[:, :], ig[:, :])
                tct = ep.tile([BC, H], BF16, tag="tct", name=f"tct{t}")
                nc.scalar.activation(tct[:, :], cst[nxt][:, :], AF.Tanh)

                jo = (t // 4) % 2
                off = 64 * (t % 4)
                ht = ep.tile([BC, H], BF16, tag="ht", name=f"ht{t}")
                pt = tps.tile([128, 4 * BC], BF16, tag="pt", name=f"pt{t}")
                for k in range(NKT):
                    nc.vector.tensor_mul(ht[:, 128 * k:128 * (k + 1)],
                                         acts4[:, 3, 128 * k:128 * (k + 1)],
                                         tct[:, 128 * k:128 * (k + 1)])
                    nc.tensor.matmul(pt[:, BC * k:BC * (k + 1)],
                                     ht[:, 128 * k:128 * (k + 1)],
                                     ident[:, :],
                                     start=(k == 0), stop=(k == NKT - 1),
                                     is_transpose=True, skip_group_check=True)
                    nc.scalar.copy(ostg[jo][:, off + BC * k:off + BC * (k + 1)],
                                   pt[:, BC * k:BC * (k + 1)])
                if t % 4 == 3:
                    nc.sync.dma_start(
                        outb_d[:, t - 3:t + 1, :].rearrange("p t n -> p (t n)"),
                        ostg[jo][:, :])

    nc.compile()
    return nc


# ---------------------------------------------------------------------------
from concourse.bass_utils import run_bass_kernel_spmd

_NC_CACHE = {}


def _get_nc():
    if "nc" not in _NC_CACHE:
        _NC_CACHE["nc"] = build_nc(n_steps=S)
    return _NC_CACHE["nc"]


def kernel(**inputs):
    nc = _get_nc()
    in_maps = prep_core_inputs(**{k: np.asarray(v) for k, v in inputs.items()})
    res = run_bass_kernel_spmd(nc, in_maps, list(range(NC)))
    return assemble_output(res.results)


# revision 11
# speedup vs baseline: 2.2702x; 2.2702x over previous
"""Bass/Tile kernel for the bidirectional LSTM (S=512, B=64, I=H=512).

Sharding v2 (dir-split): 8 cores = 2 directions x 4 batch-quarters.
Core c: direction d = f if c<4 else b, batch rows [16q : 16q+16], q = c%4.

Per core:
  Phase 1: xproj = x @ W_ih.T + b  (fp32r GEMM) -> DRAM ring, bf16,
           layout ring[step, 64, 512] where rows = chunk-major (c,b):
           gate-column chunk order (f, g, i, o), 512 cols each.
  Phase 2: 512-step recurrence, one chain per core.  All matmuls are plain
    128x128-mode, base partition 0 (no tile_position - col-tiling is a PE
    mode switch that corrupts when mixed with transpose mode).
    PSUM gates tile G [16, 4, 512] fp32 (4 banks, chunk c = bank c):
      - xs inject: matmul(G[:,c,:], eye16, xs[:,c,:], start=True) per chunk
      - 4 hh matmuls bf16 per chunk (k-tiles), accumulate, stop on k=3.
      Chunk-major order so bank c completes early and ACT pipelines behind PE.
    ACT: sigmoid(f), tanh(g), sigmoid(i), sigmoid(o) -> acts4 bf16 base 0;
         tanh(c'); 4 hT psum->sbuf copies.
    DVE: fc = f*c (fp32), ig = g*i (bf16 2x), c' = fc+ig, h = o*tanh(c') per
         128-col chunk; PE transposes h chunks into hT (outstage) for t+1.
    hT lives in outstage [128, 256] (4 steps per slot, 2 slots) -> one output
    DMA per 4 steps into outb [128, 512, 64] bf16.
"""

import sys
if "/opt/trn_rl_repo" not in sys.path:
    sys.path.insert(0, "/opt/trn_rl_repo")
import numpy as np
import ml_dtypes

import concourse.bass as bass
import concourse.bacc as bacc
import concourse.mybir as mybir
import concourse.tile as tile

F32 = mybir.dt.float32
F32R = mybir.dt.float32r
BF16 = mybir.dt.bfloat16
AF = mybir.ActivationFunctionType
NP_BF16 = ml_dtypes.bfloat16

S, B, I, H = 512, 64, 512, 512
NC = 8
BC = 16               # batch rows per core
G4 = 4 * H            # 2048
NKT = 4               # K tiles of 128 over H
NCH = 4               # gate-column chunks of 512
TOK_TILE = 128        # phase-1 token tile = 8 steps
P1_LOOK = 2

# gate-column chunk order (f, g, i, o); PyTorch row order in W is (i, f, g, o)
CHUNK_GATES = (1, 2, 0, 3)
PERM = np.concatenate([np.arange(g * H, (g + 1) * H) for g in CHUNK_GATES])


def prep_core_inputs(inpt, W_ih_f, W_hh_f, b_ih_f, b_hh_f,
                     W_ih_b, W_hh_b, b_ih_b, b_hh_b):
    x_f = np.ascontiguousarray(inpt)
    x_b = np.ascontiguousarray(inpt[::-1])

    per_dir = {}
    for d, (Wih, Whh, bih, bhh) in (("f", (W_ih_f, W_hh_f, b_ih_f, b_hh_f)),
                                    ("b", (W_ih_b, W_hh_b, b_ih_b, b_hh_b))):
        Wr_ih = np.ascontiguousarray(Wih.T[:, PERM], dtype=np.float32)   # [512,2048]
        Wr_hh = np.ascontiguousarray(Whh.T[:, PERM], dtype=np.float32)
        bias = np.ascontiguousarray((bih + bhh)[PERM], dtype=np.float32)[None, :]
        per_dir[d] = {
            "Wih": np.ascontiguousarray(
                Wr_ih.reshape(4, 128, G4).transpose(1, 0, 2)),            # [128,4,2048]
            "Whh": np.ascontiguousarray(
                Wr_hh.reshape(4, 128, G4).transpose(1, 0, 2)).astype(NP_BF16),
            "bias": bias,
        }

    ident = np.eye(BC, dtype=np.float32)
    ones1 = np.ones((1, 128), dtype=np.float32)

    in_maps = []
    for core in range(NC):
        d = "f" if core < 4 else "b"
        q = core % 4
        bs = slice(BC * q, BC * (q + 1))
        x = (x_f if d == "f" else x_b)[:, bs, :]          # [S, 16, 512]
        xT = np.ascontiguousarray(
            x.reshape(S * BC, I).T, dtype=np.float32)     # [512, 8192]
        m = {
            "xT": xT,
            "Wih": per_dir[d]["Wih"],
            "Whh": per_dir[d]["Whh"],
            "bias": per_dir[d]["bias"],
            "ident": ident.astype(NP_BF16),
            "ones1": ones1,
        }
        in_maps.append(m)
    return in_maps


def assemble_output(results):
    out = np.empty((S, B, 2 * H), dtype=np.float32)
    for core in range(NC):
        d = "f" if core < 4 else "b"
        q = core % 4
        bs = slice(BC * q, BC * (q + 1))
        off = 0 if d == "f" else H
        slab = results[core]["outb"].astype(np.float32)    # [128, S, 64]
        h = slab.reshape(128, S, 4, BC).transpose(1, 3, 2, 0).reshape(S, BC, H)
        out[:, bs, off:off + H] = h
    return out


def build_nc(n_steps=S):
    assert n_steps % 8 == 0
    nc = bacc.Bacc("TRN2", target_bir_lowering=False, debug=False)
    n_tt = (n_steps * BC) // TOK_TILE          # token tiles (8 steps each)

    xT_d = nc.declare_dram_parameter("xT", [I, S * BC], F32R, isOutput=False)
    Wih_d = nc.declare_dram_parameter("Wih", [128, 4, G4], F32R, isOutput=False)
    Whh_d = nc.declare_dram_parameter("Whh", [128, 4, G4], BF16, isOutput=False)
    bias_d = nc.declare_dram_parameter("bias", [1, G4], F32R, isOutput=False)
    ident_d = nc.declare_dram_parameter("ident", [BC, BC], BF16, isOutput=False)
    ones_d = nc.declare_dram_parameter("ones1", [1, 128], F32R, isOutput=False)
    outb_d = nc.declare_dram_parameter("outb", [128, n_steps, 4 * BC], BF16,
                                       isOutput=True)
    ring = nc.dram_tensor("ring", [n_steps, BC, 4, 512], BF16)
    # scratch sink for keep-alive DMAs (see loop): one unique slot per DMA so
    # they are fully independent of each other and of the compute dataflow
    scr = nc.dram_tensor("scr", [8 * n_steps + 64, 128], F32R)

    with tile.TileContext(nc) as tc:
        with (
            tc.tile_pool(name="wpool", bufs=1) as wpool,
            tc.tile_pool(name="p1x", bufs=2) as p1x,
            tc.tile_pool(name="p1o", bufs=2) as p1o,
            tc.tile_pool(name="p1ps", bufs=2, space="PSUM") as p1ps,
            tc.tile_pool(name="xsp", bufs=4) as xsp,
            tc.tile_pool(name="gps", bufs=1, space="PSUM") as gps,
            tc.tile_pool(name="tps", bufs=2, space="PSUM") as tps,
            tc.tile_pool(name="ep", bufs=2) as ep,
            tc.tile_pool(name="hst", bufs=1) as hst,
        ):
            # resident weights / constants
            Whh = wpool.tile([128, 4, G4], BF16, tag="whh")
            Wih = wpool.tile([128, 4, G4], F32R, tag="wih")
            for k in range(NKT):
                nc.sync.dma_start(Whh[:, k, :], Whh_d[:, k, :])
                nc.sync.dma_start(Wih[:, k, :], Wih_d[:, k, :])
            bias = wpool.tile([1, G4], F32R, tag="bias")
            ident = wpool.tile([BC, BC], BF16, tag="ident")
            ones1 = wpool.tile([1, 128], F32R, tag="ones")
            nc.sync.dma_start(bias[:, :], bias_d[:, :])
            nc.sync.dma_start(ident[:, :], ident_d[:, :])
            nc.sync.dma_start(ones1[:, :], ones_d[:, :])

            # state
            hT0 = hst.tile([128, 4 * BC], BF16, tag="hT0")      # zeros for t=0
            nc.vector.memset(hT0[:, :].bitcast(F32), 0.0)
            ostg = [hst.tile([128, 4 * 4 * BC], BF16, tag=f"ostg{j}",
                             name=f"ostg{j}") for j in range(2)]
            cst = [hst.tile([BC, H], F32, tag=f"cst{j}", name=f"cst{j}")
                   for j in range(2)]
            nc.vector.memset(cst[0][:, :], 0.0)

            xTd = xT_d.rearrange("(k p) t -> p k t", p=128)

            def emit_p1_tile(i):
                xt = p1x.tile([128, 4, TOK_TILE], F32R, tag="xt", name=f"xt{i}")
                nc.sync.dma_start(xt[:, :, :],
                                  xTd[:, :, i * TOK_TILE:(i + 1) * TOK_TILE])
                for c in range(NCH):
                    ps = p1ps.tile([128, 512], F32, tag="p1ps", name=f"p1ps{i}{c}")
                    for k in range(NKT):
                        nc.tensor.matmul(ps[:, :], xt[:, k, :],
                                         Wih[:, k, 512 * c:512 * (c + 1)],
                                         start=(k == 0), stop=False)
                    nc.tensor.matmul(ps[:, :], ones1[:, :],
                                     bias[:, 512 * c:512 * (c + 1)],
                                     start=False, stop=True)
                    xo = p1o.tile([128, 512], BF16, tag="p1o", name=f"p1o{i}{c}")
                    nc.scalar.copy(xo[:, :], ps[:, :])
                    nc.sync.dma_start(
                        ring[8 * i:8 * (i + 1), :, c, :],
                        xo[:, :])

            for i in range(min(P1_LOOK, n_tt)):
                emit_p1_tile(i)

            for t in range(n_steps):
                cur, nxt = t % 2, (t + 1) % 2
                # keep-alive DMAs: tiny, independent writes that keep the DMA
                # subsystem busy; they never gate compute (read-only ones1 src,
                # unique dram slot each). 4 via SP (HWDGE) + 4 via Pool (SWDGE).
                for j in range(6):
                    nc.sync.dma_start(scr[8 * t + j:8 * t + j + 1, :],
                                      ones1[:, :])
                if t % 8 == 0:
                    nxt_tile = t // 8 + P1_LOOK
                    if nxt_tile < n_tt:
                        emit_p1_tile(nxt_tile)

                # previous h (transposed), 64-wide slice of an outstage slot
                if t == 0:
                    hTp = hT0
                    hoff = 0
                else:
                    pj = ((t - 1) // 4) % 2
                    hTp = ostg[pj]
                    hoff = 64 * ((t - 1) % 4)

                xs = xsp.tile([BC, 4, 512], BF16, tag="xs", name=f"xs{t}")
                nc.sync.dma_start(xs[:, :, :], ring[t, :, :, :])

                G = gps.tile([BC, 4, 512], F32, tag="G", name=f"G{t}")
                for c in range(NCH):
                    nc.tensor.matmul(G[:, c, :], ident[:, :], xs[:, c, :],
                                     start=True, stop=False,
                                     skip_group_check=True)
                    for k in range(NKT):
                        nc.tensor.matmul(
                            G[:, c, :],
                            hTp[:, hoff + BC * k:hoff + BC * (k + 1)],
                            Whh[:, k, 512 * c:512 * (c + 1)],
                            start=False, stop=(k == NKT - 1),
                            skip_group_check=True)

                # gate activations: acts4[:, j, :] = f, g, i, o for j = 0..3
                acts4 = ep.tile([BC, 4, 512], BF16, tag="acts", name=f"acts{t}")
                nc.scalar.activation(acts4[:, 0, :], G[:, 0, :], AF.Sigmoid)
                nc.scalar.activation(acts4[:, 1, :], G[:, 1, :], AF.Tanh)
                nc.scalar.activation(acts4[:, 2, :], G[:, 2, :], AF.Sigmoid)
                nc.scalar.activation(acts4[:, 3, :], G[:, 3, :], AF.Sigmoid)

                fc = ep.tile([BC, H], F32, tag="fc", name=f"fc{t}")
                nc.vector.tensor_mul(fc[:, :], acts4[:, 0, :], cst[cur][:, :])
                ig = ep.tile([BC, H], BF16, tag="ig", name=f"ig{t}")
                nc.vector.tensor_mul(ig[:, :], acts4[:, 1, :], acts4[:, 2, :])
                nc.vector.tensor_add(cst[nxt][:, :], fc[:, :], ig[:, :])
                tct = ep.tile([BC, H], BF16, tag="tct", name=f"tct{t}")
                nc.scalar.activation(tct[:, :], cst[nxt][:, :], AF.Tanh)

                jo = (t // 4) % 2
                off = 64 * (t % 4)
                ht = ep.tile([BC, H], BF16, tag="ht", name=f"ht{t}")
                pt = tps.tile([128, 4 * BC], BF16, tag="pt", name=f"pt{t}")
                for k in range(NKT):
                    nc.vector.tensor_mul(ht[:, 128 * k:128 * (k + 1)],
                                         acts4[:, 3, 128 * k:128 * (k + 1)],
                                         tct[:, 128 * k:128 * (k + 1)])
                    nc.tensor.matmul(pt[:, BC * k:BC * (k + 1)],
                                     ht[:, 128 * k:128 * (k + 1)],
                                     ident[:, :],
                                     start=(k == 0), stop=(k == NKT - 1),
                                     is_transpose=True, skip_group_check=True)
                    nc.scalar.copy(ostg[jo][:, off + BC * k:off + BC * (k + 1)],
                                   pt[:, BC * k:BC * (k + 1)])
                if t % 4 == 3:
                    nc.sync.dma_start(
                        outb_d[:, t - 3:t + 1, :].rearrange("p t n -> p (t n)"),
                        ostg[jo][:, :])

    nc.compile()
    return nc


# ---------------------------------------------------------------------------
from concourse.bass_utils import run_bass_kernel_spmd

_NC_CACHE = {}


def _get_nc():
    if "nc" not in _NC_CACHE:
        _NC_CACHE["nc"] = build_nc(n_steps=S)
    return _NC_CACHE["nc"]


def kernel(**inputs):
    nc = _get_nc()
    in_maps = prep_core_inputs(**{k: np.asarray(v) for k, v in inputs.items()})
    res = run_bass_kernel_spmd(nc, in_maps, list(range(NC)))
    return assemble_output(res.results)
